# revision 3
# baseline (speedup 1.0000x reference)
"""Causal self-attention on 8 Trainium2 NeuronCores.

Reference (fp32):
    qkv = x @ W_qkv + b_qkv ; split q,k,v ; heads H=16, Dh=64
    scores = q @ k^T / sqrt(Dh), causal mask, softmax
    out = (attn @ v) re-merged ; y = out @ W_proj + b_proj

Sharding: tensor-parallel over heads x data-parallel over batch.
Core c (0..7) owns batch b = c//4 and head group g = c%4 (heads 4g..4g+3).
Each core computes q^T,k^T,v for its 4 heads from x[b]^T, runs causal
attention (scores transposed layout, exp without max-subtraction -- scores
are O(5) so fp32 exp is safe, denominator via an appended ones-column in
the V matmul), then its partial y^T = O^T @ W_proj[rows]. The 4 cores of a
batch ReduceScatter(add) the [1024, 2048] partial y^T; each core adds its
b_proj slice and returns a [256, 2048] slice of y^T. Host reassembles.

All matmuls run as float32r (reduced-precision fp32, 4x faster than fp32
on the PE); end-to-end error vs the fp32 reference is ~2e-4 of max|y|.
"""

import numpy as np

import concourse.bacc as bacc
import concourse.mybir as mybir
import concourse.tile as tile
from concourse.bass_utils import run_bass_kernel_spmd

B = 2
T = 2048
C = 1024
H = 16
DH = 64
G = 4  # heads per core
N_CORES = 8
TQ = 512  # q-chunk width
NKT = T // 128  # k tiles per head
NJQ = T // TQ  # q chunks
SCALE = 1.0 / np.sqrt(DH)
GROUPS = [[0, 1, 2, 3], [4, 5, 6, 7]]

F32 = mybir.dt.float32
F32R = mybir.dt.float32r

_PROG = None


def _build_program():
    nc = bacc.Bacc(
        "TRN2", target_bir_lowering=False, debug=False, num_devices=N_CORES
    )
    xt_d = nc.dram_tensor("xt", [C, T], F32R, kind="ExternalInput").ap()
    wq_d = nc.dram_tensor("wq", [C, G * DH], F32R, kind="ExternalInput").ap()
    wk_d = nc.dram_tensor("wk", [C, G * DH], F32R, kind="ExternalInput").ap()
    wv_d = nc.dram_tensor("wv", [C, G * DH], F32R, kind="ExternalInput").ap()
    wp_d = nc.dram_tensor("wp", [G * DH, C], F32R, kind="ExternalInput").ap()
    bq_d = nc.dram_tensor("bq", [G * DH, 1], F32, kind="ExternalInput").ap()
    bk_d = nc.dram_tensor("bk", [G * DH, 1], F32, kind="ExternalInput").ap()
    bv_d = nc.dram_tensor("bv", [1, G * DH], F32, kind="ExternalInput").ap()
    bp_d = nc.dram_tensor("bp", [C // 4, 1], F32, kind="ExternalInput").ap()
    mask_d = nc.dram_tensor("mask", [128, 896], F32R, kind="ExternalInput").ap()
    ones_d = nc.dram_tensor("ones", [128, 64], F32R, kind="ExternalInput").ap()
    rs_in = nc.dram_tensor("rs_in", [C, T], F32).ap()
    rs_out = nc.dram_tensor("rs_out", [C // 4, T], F32).ap()
    y_d = nc.dram_tensor("y", [C // 4, T], F32, kind="ExternalOutput").ap()

    NCK = C // 128  # contraction tiles over model dim

    with tile.TileContext(nc) as tc:
        with (
            nc.allow_low_precision(reason="float32r matmul pipeline by design"),
            tc.tile_pool(name="ll", bufs=1) as ll,
            tc.tile_pool(name="mmps", bufs=3, space="PSUM") as mmps,
            tc.tile_pool(name="sps", bufs=2, space="PSUM") as spsp,
            tc.tile_pool(name="ovps", bufs=2, space="PSUM") as ovpsp,
            tc.tile_pool(name="rbps", bufs=1, space="PSUM") as rbpsp,
        ):
            # ---- long-lived tiles -------------------------------------
            qT = [ll.tile([128, T], F32R, tag=f"qT{p}", name=f"qT{p}") for p in range(2)]
            kT = [ll.tile([128, T], F32R, tag=f"kT{p}", name=f"kT{p}") for p in range(2)]
            oT = [ll.tile([128, T], F32R, tag=f"oT{p}", name=f"oT{p}") for p in range(2)]
            # v for 4 heads + ones col per head: [128, 4*65]
            vaug = [ll.tile([128, G * 65], F32R, tag=f"va{t}", name=f"va{t}") for t in range(NKT)]
            mask = ll.tile([128, 896], F32R, tag="mask")
            nc.sync.dma_start(out=mask[:], in_=mask_d[:])
            ones_sb = ll.tile([128, 64], F32R, tag="ones")
            nc.sync.dma_start(out=ones_sb[:], in_=ones_d[:])
            wp_sb = [ll.tile([128, C], F32R, tag=f"wp{p}", name=f"wp{p}") for p in range(2)]
            for p in range(2):
                nc.sync.dma_start(
                    out=wp_sb[p][:], in_=wp_d[p * 128 : (p + 1) * 128, :]
                )
            bq_sb = [ll.tile([128, 1], F32, tag=f"bq{p}", name=f"bq{p}") for p in range(2)]
            bk_sb = [ll.tile([128, 1], F32, tag=f"bk{p}", name=f"bk{p}") for p in range(2)]
            for p in range(2):
                nc.sync.dma_start(
                    out=bq_sb[p][:], in_=bq_d[p * 128 : (p + 1) * 128, :]
                )
                nc.sync.dma_start(
                    out=bk_sb[p][:], in_=bk_d[p * 128 : (p + 1) * 128, :]
                )
            bv_sb = ll.tile([1, G * DH], F32, tag="bv")
            nc.sync.dma_start(out=bv_sb[:], in_=bv_d[:])
            bp_sb = [ll.tile([128, 1], F32, tag=f"bp{i}", name=f"bp{i}") for i in range(2)]
            for i in range(2):
                nc.sync.dma_start(
                    out=bp_sb[i][:], in_=bp_d[i * 128 : (i + 1) * 128, :]
                )
            # broadcast bv across partitions once: ones[:,0:1]... via matmul
            bvb_ps = rbpsp.tile([128, G * DH], F32, tag="rb")
            bvb_sb = ll.tile([128, G * DH], F32, tag="bvb")
            # lhsT = ones col [1,128]? need ones row [1, 128] for M=128
            ones_row = ll.tile([1, 128], F32R, tag="ones_row")
            nc.sync.dma_start(out=ones_row[:, 0:64], in_=ones_d[0:1, :])
            nc.sync.dma_start(out=ones_row[:, 64:128], in_=ones_d[0:1, :])
            bv_r = ll.tile([1, G * DH], F32R, tag="bvr")
            nc.vector.tensor_copy(out=bv_r[:], in_=bv_sb[:])
            nc.tensor.matmul(
                bvb_ps[:], lhsT=ones_row[:], rhs=bv_r[:], start=True, stop=True
            )
            nc.vector.tensor_copy(out=bvb_sb[:], in_=bvb_ps[:])

            # ---- phase A: qkv projections -----------------------------
            with tc.tile_pool(name="pa", bufs=1) as pa:
                xt_sb = []
                for k in range(NCK):
                    t = pa.tile([128, T], F32R, tag=f"xt{k}")
                    nc.sync.dma_start(
                        out=t[:], in_=xt_d[k * 128 : (k + 1) * 128, :]
                    )
                    xt_sb.append(t)
                wq_sb, wk_sb, wv_sb = [], [], []
                for k in range(NCK):
                    for name, dst, src in (
                        ("q", wq_sb, wq_d),
                        ("k", wk_sb, wk_d),
                        ("v", wv_sb, wv_d),
                    ):
                        t = pa.tile([128, G * DH], F32R, tag=f"w{name}{k}")
                        nc.sync.dma_start(
                            out=t[:], in_=src[k * 128 : (k + 1) * 128, :]
                        )
                        dst.append(t)

                # q^T and k^T, head-pair stacked [128, T]
                for which, wsb, bsb, dst in (
                    ("q", wq_sb, bq_sb, qT),
                    ("k", wk_sb, bk_sb, kT),
                ):
                    for p in range(2):
                        for j in range(NJQ):
                            ps = mmps.tile([128, TQ], F32, tag="mm")
                            for k in range(NCK):
                                nc.tensor.matmul(
                                    ps[:],
                                    lhsT=wsb[k][:, p * 128 : (p + 1) * 128],
                                    rhs=xt_sb[k][:, j * TQ : (j + 1) * TQ],
                                    start=(k == 0),
                                    stop=(k == NCK - 1),
                                )
                            nc.vector.tensor_scalar_add(
                                out=dst[p][:, j * TQ : (j + 1) * TQ],
                                in0=ps[:],
                                scalar1=bsb[p][:],
                            )
                # v natural layout into vaug tiles
                for t in range(NKT):
                    ps = mmps.tile([128, G * DH], F32, tag="mm")
                    for k in range(NCK):
                        nc.tensor.matmul(
                            ps[:],
                            lhsT=xt_sb[k][:, t * 128 : (t + 1) * 128],
                            rhs=wv_sb[k][:],
                            start=(k == 0),
                            stop=(k == NCK - 1),
                        )
                    va = vaug[t].rearrange("p (h x) -> p h x", x=65)
                    nc.vector.tensor_add(
                        out=va[:, :, 0:64],
                        in0=ps[:].rearrange("p (h x) -> p h x", x=64),
                        in1=bvb_sb[:].rearrange("p (h x) -> p h x", x=64),
                    )
                    nc.sync.dma_start(
                        out=va[:, :, 64:65],
                        in_=ones_d[:, 0:G].rearrange("p (h x) -> p h x", x=1),
                    )

            # ---- phases B+C: attention + projection -------------------
            with (
                tc.tile_pool(name="es", bufs=4) as esp,
                tc.tile_pool(name="wk2", bufs=4) as wk2,
                tc.tile_pool(name="oc", bufs=4) as ocp,
                tc.tile_pool(name="rsy", bufs=2) as rsyp,
            ):
                for h in range(G):
                    p, r = h // 2, 64 * (h % 2)
                    qh = qT[p][r : r + 64, :]
                    kh = kT[p][r : r + 64, :]
                    for jq in range(NJQ):
                        kmax = 4 * jq + 4
                        ovps = ovpsp.tile([65, TQ], F32, tag="ov")
                        es_tiles = []
                        for kt in range(kmax):
                            sps = spsp.tile([128, TQ], F32, tag="s")
                            nc.tensor.matmul(
                                sps[:],
                                lhsT=kh[:, kt * 128 : (kt + 1) * 128],
                                rhs=qh[:, jq * TQ : (jq + 1) * TQ],
                                start=True,
                                stop=True,
                            )
                            es = esp.tile([128, TQ], F32R, tag="es")
                            nc.scalar.activation(
                                out=es[:],
                                in_=sps[:],
                                func=mybir.ActivationFunctionType.Exp,
                                scale=SCALE,
                            )
                            if kt >= 4 * jq:  # diagonal tile: causal mask
                                off = 384 - (128 * kt - TQ * jq)
                                nc.vector.tensor_mul(
                                    out=es[:],
                                    in0=es[:],
                                    in1=mask[:, off : off + TQ],
                                )
                            es_tiles.append(es)
                            nc.tensor.matmul(
                                ovps[:],
                                lhsT=vaug[kt].rearrange(
                                    "p (h x) -> p h x", x=65
                                )[:, h, :],
                                rhs=es[:],
                                start=(kt == 0),
                                stop=(kt == kmax - 1),
                            )
                        rec = wk2.tile([1, TQ], F32R, tag="rec")
                        nc.vector.reciprocal(out=rec[:], in_=ovps[64:65, :])
                        recb = rbpsp.tile([64, TQ], F32, tag="rb")
                        nc.tensor.matmul(
                            recb[:],
                            lhsT=ones_row[0:1, 0:64],
                            rhs=rec[:],
                            start=True,
                            stop=True,
                        )
                        dst = oT[p][r : r + 64, jq * TQ : (jq + 1) * TQ]
                        nc.vector.tensor_copy(out=dst, in_=ovps[0:64, :])
                        nc.vector.tensor_mul(out=dst, in0=dst, in1=recb[:])

                # projection: y^T partial = wp^T-slices @ oT
                for et in range(C // 128):
                    for j in range(NJQ):
                        ps = mmps.tile([128, TQ], F32, tag="mm")
                        for p in range(2):
                            nc.tensor.matmul(
                                ps[:],
                                lhsT=wp_sb[p][:, et * 128 : (et + 1) * 128],
                                rhs=oT[p][:, j * TQ : (j + 1) * TQ],
                                start=(p == 0),
                                stop=(p == 1),
                            )
                        o = ocp.tile([128, TQ], F32, tag="oc")
                        nc.vector.tensor_copy(out=o[:], in_=ps[:])
                        nc.sync.dma_start(
                            out=rs_in[
                                et * 128 : (et + 1) * 128, j * TQ : (j + 1) * TQ
                            ],
                            in_=o[:],
                        )

                # ---- phase D: reduce across the 4 cores of this batch --
                nc.gpsimd.collective_compute(
                    "ReduceScatter",
                    mybir.AluOpType.add,
                    ins=[rs_in[:]],
                    outs=[rs_out[:]],
                    replica_groups=GROUPS,
                )
                for i in range(2):
                    t = rsyp.tile([128, T], F32, tag="rs")
                    nc.sync.dma_start(
                        out=t[:], in_=rs_out[i * 128 : (i + 1) * 128, :]
                    )
                    nc.vector.tensor_scalar_add(
                        out=t[:], in0=t[:], scalar1=bp_sb[i][:]
                    )
                    nc.sync.dma_start(
                        out=y_d[i * 128 : (i + 1) * 128, :], in_=t[:]
                    )

    nc.compile()
    return nc


def _get_program():
    global _PROG
    if _PROG is None:
        _PROG = _build_program()
    return _PROG


def kernel(x, W_qkv, b_qkv, W_proj, b_proj):
    x = np.asarray(x, dtype=np.float32)
    W_qkv = np.asarray(W_qkv, dtype=np.float32)
    b_qkv = np.asarray(b_qkv, dtype=np.float32)
    W_proj = np.asarray(W_proj, dtype=np.float32)
    b_proj = np.asarray(b_proj, dtype=np.float32)

    nc = _get_program()

    u = np.arange(896)[None, :]
    kl = np.arange(128)[:, None]
    mask_host = (u >= kl + 384).astype(np.float32)
    ones_host = np.ones((128, 64), dtype=np.float32)

    xts = [np.ascontiguousarray(x[b].T) for b in range(B)]
    in_maps = []
    for c in range(N_CORES):
        b, g = divmod(c, 4)
        cs = slice(g * G * DH, (g + 1) * G * DH)
        in_maps.append(
            {
                "xt": xts[b],
                "wq": np.ascontiguousarray(W_qkv[:, cs]),
                "wk": np.ascontiguousarray(W_qkv[:, C:][:, cs]),
                "wv": np.ascontiguousarray(W_qkv[:, 2 * C :][:, cs]),
                "wp": np.ascontiguousarray(W_proj[cs, :]),
                "bq": np.ascontiguousarray(b_qkv[cs]).reshape(-1, 1),
                "bk": np.ascontiguousarray(b_qkv[C:][cs]).reshape(-1, 1),
                "bv": np.ascontiguousarray(b_qkv[2 * C :][cs]).reshape(1, -1),
                "bp": np.ascontiguousarray(
                    b_proj[g * (C // 4) : (g + 1) * (C // 4)]
                ).reshape(-1, 1),
                "mask": mask_host,
                "ones": ones_host,
            }
        )

    global _last_in_maps
    _last_in_maps = in_maps
    res = run_bass_kernel_spmd(nc, in_maps, list(range(N_CORES)))

    y = np.empty((B, T, C), dtype=np.float32)
    for b in range(B):
        yT = np.concatenate(
            [res.results[4 * b + r]["y"] for r in range(4)], axis=0
        )
        y[b] = yT.T
    return y


# revision 8
# speedup vs baseline: 1.0492x; 1.0492x over previous
"""Causal self-attention on 8 Trainium2 NeuronCores.

Reference (fp32):
    qkv = x @ W_qkv + b_qkv ; split q,k,v ; heads H=16, Dh=64
    scores = q @ k^T / sqrt(Dh), causal mask, softmax
    out = (attn @ v) re-merged ; y = out @ W_proj + b_proj

Sharding: tensor-parallel over heads x data-parallel over batch.
Core c (0..7) owns batch b = c//4 and head group g = c%4 (heads 4g..4g+3).
Each core computes q^T,k^T,v for its 4 heads from x[b]^T, runs causal
attention (scores transposed layout, exp without max-subtraction -- scores
are O(5) so fp32 exp is safe, denominator via an appended ones-column in
the V matmul), then its partial y^T = O^T @ W_proj[rows]. The 4 cores of a
batch ReduceScatter(add) the [1024, 2048] partial y^T in 4 row chunks
overlapped with the projection; each core adds its b_proj slice and
returns 4 x [64, 2048] row-slices of y^T. Host reassembles.

Matmuls run as float32r (reduced-precision fp32, 4x faster than fp32 on
the PE); end-to-end error vs the fp32 reference is ~3e-4 of max|y|.
The two heads of a pair occupy PE rows 0:64 / 64:128 so their score
matmuls execute concurrently in disjoint row groups.
"""

import numpy as np

import concourse.bacc as bacc
import concourse.mybir as mybir
import concourse.tile as tile
from concourse.bass_utils import run_bass_kernel_spmd

B = 2
T = 2048
C = 1024
H = 16
DH = 64
G = 4  # heads per core
N_CORES = 8
TQ = 512  # q-chunk width
NKT = T // 128  # k tiles per head
NJQ = T // TQ  # q chunks
NCK = C // 128  # contraction tiles over model dim
SCALE = 1.0 / np.sqrt(DH)
GROUPS = [[0, 1, 2, 3], [4, 5, 6, 7]]

F32 = mybir.dt.float32
F32R = mybir.dt.float32r

_PROG = None


def _build_program():
    nc = bacc.Bacc(
        "TRN2", target_bir_lowering=False, debug=False, num_devices=N_CORES
    )
    xt_d = nc.dram_tensor("xt", [C, T], F32R, kind="ExternalInput").ap()
    wq_d = nc.dram_tensor("wq", [C, G * DH], F32R, kind="ExternalInput").ap()
    wk_d = nc.dram_tensor("wk", [C, G * DH], F32R, kind="ExternalInput").ap()
    wv_d = nc.dram_tensor("wv", [C, G * DH], F32R, kind="ExternalInput").ap()
    wp_d = nc.dram_tensor("wp", [G * DH, C], F32R, kind="ExternalInput").ap()
    bq_d = nc.dram_tensor("bq", [G * DH, 1], F32, kind="ExternalInput").ap()
    bk_d = nc.dram_tensor("bk", [G * DH, 1], F32, kind="ExternalInput").ap()
    bv_d = nc.dram_tensor("bv", [1, G * DH], F32, kind="ExternalInput").ap()
    bp_d = nc.dram_tensor("bp", [C // 4, 1], F32, kind="ExternalInput").ap()
    mask_d = nc.dram_tensor("mask", [128, 896], F32R, kind="ExternalInput").ap()
    ones_d = nc.dram_tensor("ones", [128, 64], F32R, kind="ExternalInput").ap()
    bc2_d = nc.dram_tensor("bc2", [2, 128], F32R, kind="ExternalInput").ap()
    rs_in = nc.dram_tensor("rs_in", [C, T], F32).ap()
    rs_out = nc.dram_tensor("rs_out", [C // 4, T], F32).ap()
    y_d = nc.dram_tensor("y", [C // 4, T], F32, kind="ExternalOutput").ap()

    with tile.TileContext(nc) as tc:
        with (
            nc.allow_low_precision(reason="float32r matmul pipeline by design"),
            tc.tile_pool(name="ll", bufs=1) as ll,
        ):
            # ---- long-lived tiles -------------------------------------
            qT = [ll.tile([128, T], F32R, tag=f"qT{p}", name=f"qT{p}") for p in range(2)]
            kT = [ll.tile([128, T], F32R, tag=f"kT{p}", name=f"kT{p}") for p in range(2)]
            oT = [ll.tile([128, T], F32R, tag=f"oT{p}", name=f"oT{p}") for p in range(2)]
            vaug = [ll.tile([128, G * 65], F32R, tag=f"va{t}", name=f"va{t}") for t in range(NKT)]
            den16 = ll.tile([16, TQ], F32, tag="den16")
            rec16 = ll.tile([16, TQ], F32R, tag="rec16")

            mask = ll.tile([128, 896], F32R, tag="mask")
            nc.sync.dma_start(out=mask[:], in_=mask_d[:])
            bc2_sb = ll.tile([2, 128], F32R, tag="bc2")
            nc.sync.dma_start(out=bc2_sb[:], in_=bc2_d[:])
            ones_sb = ll.tile([128, 64], F32R, tag="ones")
            nc.sync.dma_start(out=ones_sb[:], in_=ones_d[:])
            wp_sb = [ll.tile([128, C], F32R, tag=f"wp{p}", name=f"wp{p}") for p in range(2)]
            for p in range(2):
                nc.sync.dma_start(
                    out=wp_sb[p][:], in_=wp_d[p * 128 : (p + 1) * 128, :]
                )
            bq_sb = [ll.tile([128, 1], F32, tag=f"bq{p}", name=f"bq{p}") for p in range(2)]
            bk_sb = [ll.tile([128, 1], F32, tag=f"bk{p}", name=f"bk{p}") for p in range(2)]
            for p in range(2):
                nc.sync.dma_start(
                    out=bq_sb[p][:], in_=bq_d[p * 128 : (p + 1) * 128, :]
                )
                nc.sync.dma_start(
                    out=bk_sb[p][:], in_=bk_d[p * 128 : (p + 1) * 128, :]
                )
            bv_sb = ll.tile([1, G * DH], F32, tag="bv")
            nc.sync.dma_start(out=bv_sb[:], in_=bv_d[:])
            bp_sb = [ll.tile([128, 1], F32, tag=f"bp{i}", name=f"bp{i}") for i in range(2)]
            for i in range(2):
                nc.sync.dma_start(
                    out=bp_sb[i][:], in_=bp_d[i * 128 : (i + 1) * 128, :]
                )

            # ---- phase A: qkv projections -----------------------------
            with (
                tc.tile_pool(name="pa", bufs=1) as pa,
                tc.tile_pool(name="pamm", bufs=3, space="PSUM") as pamm,
            ):
                # bv broadcast across partitions (via ones-row matmul)
                ones_row = ll.tile([1, 128], F32R, tag="ones_row")
                nc.sync.dma_start(out=ones_row[:, 0:64], in_=ones_d[0:1, :])
                nc.sync.dma_start(out=ones_row[:, 64:128], in_=ones_d[0:1, :])
                bv_r = ll.tile([1, G * DH], F32R, tag="bvr")
                nc.vector.tensor_copy(out=bv_r[:], in_=bv_sb[:])
                bvb_ps = pamm.tile([128, G * DH], F32, tag="mm")
                bvb_sb = ll.tile([128, G * DH], F32, tag="bvb")
                nc.tensor.matmul(
                    bvb_ps[:], lhsT=ones_row[:], rhs=bv_r[:], start=True, stop=True
                )
                nc.vector.tensor_copy(out=bvb_sb[:], in_=bvb_ps[:])

                wq_sb, wk_sb, wv_sb = [], [], []
                for k in range(NCK):
                    for name, dst, src in (
                        ("q", wq_sb, wq_d),
                        ("k", wk_sb, wk_d),
                        ("v", wv_sb, wv_d),
                    ):
                        t = pa.tile([128, G * DH], F32R, tag=f"w{name}{k}", name=f"w{name}{k}")
                        nc.sync.dma_start(
                            out=t[:], in_=src[k * 128 : (k + 1) * 128, :]
                        )
                        dst.append(t)
                xt_sb = [
                    pa.tile([128, T], F32R, tag=f"xt{k}", name=f"xt{k}")
                    for k in range(NCK)
                ]
                for j in range(NJQ):
                    for k in range(NCK):
                        nc.sync.dma_start(
                            out=xt_sb[k][:, j * TQ : (j + 1) * TQ],
                            in_=xt_d[k * 128 : (k + 1) * 128, j * TQ : (j + 1) * TQ],
                        )

                for j in range(NJQ):
                    # q^T / k^T chains for this column chunk
                    for wsb, bsb, dst in ((wq_sb, bq_sb, qT), (wk_sb, bk_sb, kT)):
                        for p in range(2):
                            ps = pamm.tile([128, TQ], F32, tag="mm")
                            for k in range(NCK):
                                nc.tensor.matmul(
                                    ps[:],
                                    lhsT=wsb[k][:, p * 128 : (p + 1) * 128],
                                    rhs=xt_sb[k][:, j * TQ : (j + 1) * TQ],
                                    start=(k == 0),
                                    stop=(k == NCK - 1),
                                )
                            nc.vector.tensor_scalar_add(
                                out=dst[p][:, j * TQ : (j + 1) * TQ],
                                in0=ps[:],
                                scalar1=bsb[p][:],
                            )
                    # v tiles covered by this column chunk
                    for t in range(4 * j, 4 * j + 4):
                        ps = pamm.tile([128, G * DH], F32, tag="mm")
                        for k in range(NCK):
                            nc.tensor.matmul(
                                ps[:],
                                lhsT=xt_sb[k][:, t * 128 : (t + 1) * 128],
                                rhs=wv_sb[k][:],
                                start=(k == 0),
                                stop=(k == NCK - 1),
                            )
                        va = vaug[t].rearrange("p (h x) -> p h x", x=65)
                        nc.vector.tensor_add(
                            out=va[:, :, 0:64],
                            in0=ps[:].rearrange("p (h x) -> p h x", x=64),
                            in1=bvb_sb[:].rearrange("p (h x) -> p h x", x=64),
                        )
                        nc.sync.dma_start(
                            out=va[:, :, 64:65],
                            in_=ones_d[:, 0:G].rearrange("p (h x) -> p h x", x=1),
                        )

            # ---- phase B: attention (head pairs packed in PE rows) ----
            with (
                tc.tile_pool(name="dt", bufs=3) as dtp,
                tc.tile_pool(name="es", bufs=6) as esp,
                tc.tile_pool(name="ps0", bufs=2, space="PSUM") as sp0,
                tc.tile_pool(name="ps1", bufs=2, space="PSUM") as sp1,
                tc.tile_pool(name="ova", bufs=2, space="PSUM") as ova,
                tc.tile_pool(name="ovb", bufs=2, space="PSUM") as ovb,
            ):
                for p in range(2):
                    for jq in range(NJQ):
                        cidx = 4 * p + jq
                        kmax = 4 * jq + 4
                        ov = [
                            ova.tile([65, TQ], F32, tag="ovA", name="ovA"),
                            ovb.tile([65, TQ], F32, tag="ovB", name="ovB"),
                        ]
                        spool = (sp0, sp1)

                        def emit_v(kt, es_pair):
                            va = vaug[kt].rearrange("p (h x) -> p h x", x=65)
                            for half in range(2):
                                nc.tensor.matmul(
                                    ov[half][:],
                                    lhsT=va[:, 2 * p + half, :],
                                    rhs=es_pair[half][:],
                                    start=(kt == 0),
                                    stop=(kt == kmax - 1),
                                )

                        prev = None
                        for kt in range(kmax):
                            es_pair = []
                            for half in range(2):
                                r = 64 * half
                                sps = spool[half].tile([128, TQ], F32, tag="s")
                                nc.tensor.matmul(
                                    sps[:],
                                    lhsT=kT[p][
                                        r : r + 64, kt * 128 : (kt + 1) * 128
                                    ],
                                    rhs=qT[p][r : r + 64, jq * TQ : (jq + 1) * TQ],
                                    start=True,
                                    stop=True,
                                )
                                es = esp.tile([128, TQ], F32R, tag="es")
                                nc.scalar.activation(
                                    out=es[:],
                                    in_=sps[:],
                                    func=mybir.ActivationFunctionType.Exp,
                                    scale=SCALE,
                                )
                                if kt >= 4 * jq:
                                    off = 384 - (128 * kt - TQ * jq)
                                    nc.vector.tensor_mul(
                                        out=es[:],
                                        in0=es[:],
                                        in1=mask[:, off : off + TQ],
                                    )
                                es_pair.append(es)
                            if prev is not None:
                                emit_v(*prev)
                            prev = (kt, es_pair)
                        emit_v(*prev)
                        # epilogue: move unnormalized O and denominators out
                        for half in range(2):
                            idx = 2 * cidx + half
                            nc.vector.tensor_copy(
                                out=oT[p][
                                    64 * half : 64 * half + 64,
                                    jq * TQ : (jq + 1) * TQ,
                                ],
                                in_=ov[half][0:64, :],
                            )
                            dt_t = dtp.tile([1, TQ], F32, tag="dt", name="dt")
                            nc.vector.tensor_copy(
                                out=dt_t[:], in_=ov[half][64:65, :]
                            )
                            nc.sync.dma_start(
                                out=den16[idx : idx + 1, :], in_=dt_t[:]
                            )

            # ---- phase B2: batched softmax normalization --------------
            with (
                tc.tile_pool(name="rp", bufs=3) as rpp,
                tc.tile_pool(name="rb", bufs=2, space="PSUM") as rbp,
            ):
                nc.vector.reciprocal(out=rec16[:], in_=den16[:])
                for p in range(2):
                    for jq in range(NJQ):
                        cidx = 4 * p + jq
                        rp_t = rpp.tile([2, TQ], F32R, tag="rp", name="rp")
                        nc.sync.dma_start(
                            out=rp_t[:],
                            in_=rec16[2 * cidx : 2 * cidx + 2, :],
                        )
                        recb = rbp.tile([128, TQ], F32, tag="rb")
                        nc.tensor.matmul(
                            recb[:],
                            lhsT=bc2_sb[:],
                            rhs=rp_t[:],
                            start=True,
                            stop=True,
                        )
                        dst = oT[p][:, jq * TQ : (jq + 1) * TQ]
                        nc.vector.tensor_mul(out=dst, in0=dst, in1=recb[:])

            # ---- phase C: projection + chunked ReduceScatter ----------
            with (
                tc.tile_pool(name="oc", bufs=4) as ocp,
                tc.tile_pool(name="pc", bufs=3, space="PSUM") as pcm,
                tc.tile_pool(name="rsy", bufs=2) as rsyp,
            ):
                for et in range(C // 128):
                    for j in range(NJQ):
                        ps = pcm.tile([128, TQ], F32, tag="mm")
                        for p in range(2):
                            nc.tensor.matmul(
                                ps[:],
                                lhsT=wp_sb[p][:, et * 128 : (et + 1) * 128],
                                rhs=oT[p][:, j * TQ : (j + 1) * TQ],
                                start=(p == 0),
                                stop=(p == 1),
                            )
                        o = ocp.tile([128, TQ], F32, tag="oc")
                        nc.vector.tensor_copy(out=o[:], in_=ps[:])
                        nc.sync.dma_start(
                            out=rs_in[
                                et * 128 : (et + 1) * 128, j * TQ : (j + 1) * TQ
                            ],
                            in_=o[:],
                        )
                    if et % 2 == 1:  # rows 256c..256c+256 complete
                        c = et // 2
                        nc.gpsimd.collective_compute(
                            "ReduceScatter",
                            mybir.AluOpType.add,
                            ins=[rs_in[c * 256 : (c + 1) * 256, :]],
                            outs=[rs_out[c * 64 : (c + 1) * 64, :]],
                            replica_groups=GROUPS,
                        )

                # ---- final: bias + output -----------------------------
                for i in range(2):
                    t = rsyp.tile([128, T], F32, tag="rs")
                    nc.sync.dma_start(
                        out=t[:], in_=rs_out[i * 128 : (i + 1) * 128, :]
                    )
                    nc.vector.tensor_scalar_add(
                        out=t[:], in0=t[:], scalar1=bp_sb[i][:]
                    )
                    nc.sync.dma_start(
                        out=y_d[i * 128 : (i + 1) * 128, :], in_=t[:]
                    )

    nc.compile()
    return nc


def _get_program():
    global _PROG
    if _PROG is None:
        _PROG = _build_program()
    return _PROG


def kernel(x, W_qkv, b_qkv, W_proj, b_proj):
    x = np.asarray(x, dtype=np.float32)
    W_qkv = np.asarray(W_qkv, dtype=np.float32)
    b_qkv = np.asarray(b_qkv, dtype=np.float32)
    W_proj = np.asarray(W_proj, dtype=np.float32)
    b_proj = np.asarray(b_proj, dtype=np.float32)

    nc = _get_program()

    u = np.arange(896)[None, :]
    kl = np.arange(128)[:, None]
    mask_host = (u >= kl + 384).astype(np.float32)
    ones_host = np.ones((128, 64), dtype=np.float32)
    bc2_host = np.zeros((2, 128), dtype=np.float32)
    bc2_host[0, 0:64] = 1.0
    bc2_host[1, 64:128] = 1.0

    xts = [np.ascontiguousarray(x[b].T) for b in range(B)]
    in_maps = []
    for c in range(N_CORES):
        b, g = divmod(c, 4)
        cs = slice(g * G * DH, (g + 1) * G * DH)
        r = c % 4
        # bias rows for the chunked-RS output layout: y row 64*ch + i
        # corresponds to global column 256*ch + 64*r + i
        bp_rows = np.concatenate(
            [b_proj[256 * ch + 64 * r : 256 * ch + 64 * r + 64] for ch in range(4)]
        )
        in_maps.append(
            {
                "xt": xts[b],
                "wq": np.ascontiguousarray(W_qkv[:, cs]),
                "wk": np.ascontiguousarray(W_qkv[:, C:][:, cs]),
                "wv": np.ascontiguousarray(W_qkv[:, 2 * C :][:, cs]),
                "wp": np.ascontiguousarray(W_proj[cs, :]),
                "bq": np.ascontiguousarray(b_qkv[cs]).reshape(-1, 1),
                "bk": np.ascontiguousarray(b_qkv[C:][cs]).reshape(-1, 1),
                "bv": np.ascontiguousarray(b_qkv[2 * C :][cs]).reshape(1, -1),
                "bp": bp_rows.reshape(-1, 1).astype(np.float32),
                "mask": mask_host,
                "ones": ones_host,
                "bc2": bc2_host,
            }
        )

    global _last_in_maps
    _last_in_maps = in_maps
    res = run_bass_kernel_spmd(nc, in_maps, list(range(N_CORES)))

    y = np.empty((B, T, C), dtype=np.float32)
    for b in range(B):
        yT = np.empty((C, T), dtype=np.float32)
        for r in range(4):
            yc = res.results[4 * b + r]["y"]  # [256, 2048]
            for ch in range(4):
                yT[256 * ch + 64 * r : 256 * ch + 64 * r + 64] = yc[
                    64 * ch : 64 * ch + 64
                ]
        y[b] = yT.T
    return y


# revision 9
# speedup vs baseline: 1.1744x; 1.1193x over previous
"""Causal self-attention on 8 Trainium2 NeuronCores.

Reference (fp32):
    qkv = x @ W_qkv + b_qkv ; split q,k,v ; heads H=16, Dh=64
    scores = q @ k^T / sqrt(Dh), causal mask, softmax
    out = (attn @ v) re-merged ; y = out @ W_proj + b_proj

Sharding: tensor-parallel over heads x data-parallel over batch.
Core c (0..7) owns batch b = c//4 and head group g = c%4 (heads 4g..4g+3).
Each core computes q^T,k^T,v for its 4 heads from x[b]^T, runs causal
attention (scores transposed layout, exp without max-subtraction -- scores
are O(5) so fp32 exp is safe, denominator via an appended ones-column in
the V matmul), then its partial y^T = O^T @ W_proj[rows]. The 4 cores of a
batch ReduceScatter(add) the [1024, 2048] partial y^T in 4 row chunks
overlapped with the projection; each core adds its b_proj slice and
returns 4 x [64, 2048] row-slices of y^T. Host reassembles.

Matmuls run as float32r (reduced-precision fp32, 4x faster than fp32 on
the PE); end-to-end error vs the fp32 reference is ~3e-4 of max|y|.
The two heads of a pair occupy PE rows 0:64 / 64:128 so their score
matmuls execute concurrently in disjoint row groups.
"""

import numpy as np

import concourse.bacc as bacc
import concourse.mybir as mybir
import concourse.tile as tile
from concourse.bass_utils import run_bass_kernel_spmd

B = 2
T = 2048
C = 1024
H = 16
DH = 64
G = 4  # heads per core
N_CORES = 8
TQ = 512  # q-chunk width
NKT = T // 128  # k tiles per head
NJQ = T // TQ  # q chunks
NCK = C // 128  # contraction tiles over model dim
SCALE = 1.0 / np.sqrt(DH)
GROUPS = [[0, 1, 2, 3], [4, 5, 6, 7]]

F32 = mybir.dt.float32
F32R = mybir.dt.float32r

_PROG = None


def _build_program():
    nc = bacc.Bacc(
        "TRN2", target_bir_lowering=False, debug=False, num_devices=N_CORES
    )
    xt_d = nc.dram_tensor("xt", [C, T], F32R, kind="ExternalInput").ap()
    wq_d = nc.dram_tensor("wq", [C, G * DH], F32R, kind="ExternalInput").ap()
    wk_d = nc.dram_tensor("wk", [C, G * DH], F32R, kind="ExternalInput").ap()
    wv_d = nc.dram_tensor("wv", [C, G * DH], F32R, kind="ExternalInput").ap()
    wp_d = nc.dram_tensor("wp", [G * DH, C], F32R, kind="ExternalInput").ap()
    bq_d = nc.dram_tensor("bq", [G * DH, 1], F32, kind="ExternalInput").ap()
    bk_d = nc.dram_tensor("bk", [G * DH, 1], F32, kind="ExternalInput").ap()
    bv_d = nc.dram_tensor("bv", [1, G * DH], F32, kind="ExternalInput").ap()
    bp_d = nc.dram_tensor("bp", [C // 4, 1], F32, kind="ExternalInput").ap()
    mask_d = nc.dram_tensor("mask", [128, 896], F32R, kind="ExternalInput").ap()
    ones_d = nc.dram_tensor("ones", [128, 64], F32R, kind="ExternalInput").ap()
    bc2_d = nc.dram_tensor("bc2", [2, 128], F32R, kind="ExternalInput").ap()
    rs_in = [nc.dram_tensor(f"rs_in{j}", [C, TQ], F32).ap() for j in range(NJQ)]
    rs_out = [nc.dram_tensor(f"rs_out{j}", [C // 4, TQ], F32).ap() for j in range(NJQ)]
    y_d = nc.dram_tensor("y", [C // 4, T], F32, kind="ExternalOutput").ap()

    with tile.TileContext(nc) as tc:
        with (
            nc.allow_low_precision(reason="float32r matmul pipeline by design"),
            tc.tile_pool(name="ll", bufs=1) as ll,
        ):
            # ---- long-lived tiles -------------------------------------
            qT = [ll.tile([128, T], F32R, tag=f"qT{p}", name=f"qT{p}") for p in range(2)]
            kT = [ll.tile([128, T], F32R, tag=f"kT{p}", name=f"kT{p}") for p in range(2)]
            oT = [ll.tile([128, T], F32R, tag=f"oT{p}", name=f"oT{p}") for p in range(2)]
            vaug = [ll.tile([128, G * 65], F32R, tag=f"va{t}", name=f"va{t}") for t in range(NKT)]


            mask = ll.tile([128, 896], F32R, tag="mask")
            nc.sync.dma_start(out=mask[:], in_=mask_d[:])
            bc2_sb = ll.tile([2, 128], F32R, tag="bc2")
            nc.sync.dma_start(out=bc2_sb[:], in_=bc2_d[:])
            ones_sb = ll.tile([128, 64], F32R, tag="ones")
            nc.sync.dma_start(out=ones_sb[:], in_=ones_d[:])
            wp_sb = [ll.tile([128, C], F32R, tag=f"wp{p}", name=f"wp{p}") for p in range(2)]
            for p in range(2):
                nc.sync.dma_start(
                    out=wp_sb[p][:], in_=wp_d[p * 128 : (p + 1) * 128, :]
                )
            bq_sb = [ll.tile([128, 1], F32, tag=f"bq{p}", name=f"bq{p}") for p in range(2)]
            bk_sb = [ll.tile([128, 1], F32, tag=f"bk{p}", name=f"bk{p}") for p in range(2)]
            for p in range(2):
                nc.sync.dma_start(
                    out=bq_sb[p][:], in_=bq_d[p * 128 : (p + 1) * 128, :]
                )
                nc.sync.dma_start(
                    out=bk_sb[p][:], in_=bk_d[p * 128 : (p + 1) * 128, :]
                )
            bv_sb = ll.tile([1, G * DH], F32, tag="bv")
            nc.sync.dma_start(out=bv_sb[:], in_=bv_d[:])
            bp_sb = [ll.tile([128, 1], F32, tag=f"bp{i}", name=f"bp{i}") for i in range(2)]
            for i in range(2):
                nc.sync.dma_start(
                    out=bp_sb[i][:], in_=bp_d[i * 128 : (i + 1) * 128, :]
                )

            # ---- phase A: qkv projections -----------------------------
            with (
                tc.tile_pool(name="pa", bufs=1) as pa,
                tc.tile_pool(name="pamm", bufs=3, space="PSUM") as pamm,
            ):
                # bv broadcast across partitions (via ones-row matmul)
                ones_row = ll.tile([1, 128], F32R, tag="ones_row")
                nc.sync.dma_start(out=ones_row[:, 0:64], in_=ones_d[0:1, :])
                nc.sync.dma_start(out=ones_row[:, 64:128], in_=ones_d[0:1, :])
                bv_r = ll.tile([1, G * DH], F32R, tag="bvr")
                nc.vector.tensor_copy(out=bv_r[:], in_=bv_sb[:])
                bvb_ps = pamm.tile([128, G * DH], F32, tag="mm")
                bvb_sb = ll.tile([128, G * DH], F32, tag="bvb")
                nc.tensor.matmul(
                    bvb_ps[:], lhsT=ones_row[:], rhs=bv_r[:], start=True, stop=True
                )
                nc.vector.tensor_copy(out=bvb_sb[:], in_=bvb_ps[:])

                wq_sb, wk_sb, wv_sb = [], [], []
                for k in range(NCK):
                    for name, dst, src in (
                        ("q", wq_sb, wq_d),
                        ("k", wk_sb, wk_d),
                        ("v", wv_sb, wv_d),
                    ):
                        t = pa.tile([128, G * DH], F32R, tag=f"w{name}{k}", name=f"w{name}{k}")
                        nc.sync.dma_start(
                            out=t[:], in_=src[k * 128 : (k + 1) * 128, :]
                        )
                        dst.append(t)
                xt_sb = [
                    pa.tile([128, T], F32R, tag=f"xt{k}", name=f"xt{k}")
                    for k in range(NCK)
                ]
                for j in range(NJQ):
                    for k in range(NCK):
                        nc.sync.dma_start(
                            out=xt_sb[k][:, j * TQ : (j + 1) * TQ],
                            in_=xt_d[k * 128 : (k + 1) * 128, j * TQ : (j + 1) * TQ],
                        )

                for j in range(NJQ):
                    # q^T / k^T chains for this column chunk
                    for wsb, bsb, dst in ((wq_sb, bq_sb, qT), (wk_sb, bk_sb, kT)):
                        for p in range(2):
                            ps = pamm.tile([128, TQ], F32, tag="mm")
                            for k in range(NCK):
                                nc.tensor.matmul(
                                    ps[:],
                                    lhsT=wsb[k][:, p * 128 : (p + 1) * 128],
                                    rhs=xt_sb[k][:, j * TQ : (j + 1) * TQ],
                                    start=(k == 0),
                                    stop=(k == NCK - 1),
                                )
                            nc.vector.tensor_scalar_add(
                                out=dst[p][:, j * TQ : (j + 1) * TQ],
                                in0=ps[:],
                                scalar1=bsb[p][:],
                            )
                    # v tiles covered by this column chunk
                    for t in range(4 * j, 4 * j + 4):
                        ps = pamm.tile([128, G * DH], F32, tag="mm")
                        for k in range(NCK):
                            nc.tensor.matmul(
                                ps[:],
                                lhsT=xt_sb[k][:, t * 128 : (t + 1) * 128],
                                rhs=wv_sb[k][:],
                                start=(k == 0),
                                stop=(k == NCK - 1),
                            )
                        va = vaug[t].rearrange("p (h x) -> p h x", x=65)
                        nc.vector.tensor_add(
                            out=va[:, :, 0:64],
                            in0=ps[:].rearrange("p (h x) -> p h x", x=64),
                            in1=bvb_sb[:].rearrange("p (h x) -> p h x", x=64),
                        )
                        nc.sync.dma_start(
                            out=va[:, :, 64:65],
                            in_=ones_d[:, 0:G].rearrange("p (h x) -> p h x", x=1),
                        )

            # ---- phases B..D: attention, normalize, projection, RS ----
            # jq-outer so that normalize/proj/ReduceScatter pipeline per
            # 512-column block while later blocks still compute.
            with (
                tc.tile_pool(name="dt", bufs=4) as dtp,
                tc.tile_pool(name="rp", bufs=2) as rpp,
                tc.tile_pool(name="es", bufs=6) as esp,
                tc.tile_pool(name="oc", bufs=3) as ocp,
                tc.tile_pool(name="rsy", bufs=2) as rsyp,
                tc.tile_pool(name="ps0", bufs=2, space="PSUM") as sp0,
                tc.tile_pool(name="ps1", bufs=2, space="PSUM") as sp1,
                tc.tile_pool(name="ov", bufs=2, space="PSUM") as ovp,
                tc.tile_pool(name="px", bufs=2, space="PSUM") as pxp,
            ):
                for jq in range(NJQ):
                    kmax = 4 * jq + 4
                    den4 = dtp.tile([4, TQ], F32, tag="den4", name="den4")
                    for p in range(2):
                        ov = [
                            ovp.tile([65, TQ], F32, tag="ov", name="ovA"),
                            ovp.tile([65, TQ], F32, tag="ov", name="ovB"),
                        ]
                        spool = (sp0, sp1)

                        def emit_v(kt, es_pair):
                            va = vaug[kt].rearrange("p (h x) -> p h x", x=65)
                            for half in range(2):
                                nc.tensor.matmul(
                                    ov[half][:],
                                    lhsT=va[:, 2 * p + half, :],
                                    rhs=es_pair[half][:],
                                    start=(kt == 0),
                                    stop=(kt == kmax - 1),
                                )

                        prev = None
                        for kt in range(kmax):
                            es_pair = []
                            for half in range(2):
                                r = 64 * half
                                sps = spool[half].tile(
                                    [128, TQ], F32, tag="s", name="sps"
                                )
                                nc.tensor.matmul(
                                    sps[:],
                                    lhsT=kT[p][
                                        r : r + 64, kt * 128 : (kt + 1) * 128
                                    ],
                                    rhs=qT[p][r : r + 64, jq * TQ : (jq + 1) * TQ],
                                    start=True,
                                    stop=True,
                                )
                                es = esp.tile([128, TQ], F32R, tag="es", name="es")
                                nc.scalar.activation(
                                    out=es[:],
                                    in_=sps[:],
                                    func=mybir.ActivationFunctionType.Exp,
                                    scale=SCALE,
                                )
                                if kt >= 4 * jq:
                                    off = 384 - (128 * kt - TQ * jq)
                                    nc.vector.tensor_mul(
                                        out=es[:],
                                        in0=es[:],
                                        in1=mask[:, off : off + TQ],
                                    )
                                es_pair.append(es)
                            if prev is not None:
                                emit_v(*prev)
                            prev = (kt, es_pair)
                        emit_v(*prev)
                        # epilogue: move unnormalized O and denominators out
                        for half in range(2):
                            nc.vector.tensor_copy(
                                out=oT[p][
                                    64 * half : 64 * half + 64,
                                    jq * TQ : (jq + 1) * TQ,
                                ],
                                in_=ov[half][0:64, :],
                            )
                            dt_t = dtp.tile([1, TQ], F32, tag="dt", name="dt")
                            nc.vector.tensor_copy(
                                out=dt_t[:], in_=ov[half][64:65, :]
                            )
                            nc.sync.dma_start(
                                out=den4[2 * p + half : 2 * p + half + 1, :],
                                in_=dt_t[:],
                            )

                    # normalize this column block (both pairs)
                    rec4 = dtp.tile([4, TQ], F32R, tag="rec4", name="rec4")
                    nc.vector.reciprocal(out=rec4[:], in_=den4[:])
                    for p in range(2):
                        rp_t = rpp.tile([2, TQ], F32R, tag="rp", name="rp")
                        nc.sync.dma_start(
                            out=rp_t[:], in_=rec4[2 * p : 2 * p + 2, :]
                        )
                        recb = pxp.tile([128, TQ], F32, tag="x", name="recb")
                        nc.tensor.matmul(
                            recb[:],
                            lhsT=bc2_sb[:],
                            rhs=rp_t[:],
                            start=True,
                            stop=True,
                        )
                        dst = oT[p][:, jq * TQ : (jq + 1) * TQ]
                        nc.vector.tensor_mul(out=dst, in0=dst, in1=recb[:])

                    # projection for this column block
                    for et in range(C // 128):
                        ps = pxp.tile([128, TQ], F32, tag="x", name="pmm")
                        for p in range(2):
                            nc.tensor.matmul(
                                ps[:],
                                lhsT=wp_sb[p][:, et * 128 : (et + 1) * 128],
                                rhs=oT[p][:, jq * TQ : (jq + 1) * TQ],
                                start=(p == 0),
                                stop=(p == 1),
                            )
                        o = ocp.tile([128, TQ], F32, tag="oc", name="oc")
                        nc.vector.tensor_copy(out=o[:], in_=ps[:])
                        nc.sync.dma_start(
                            out=rs_in[jq][et * 128 : (et + 1) * 128, :], in_=o[:]
                        )
                    # reduce-scatter this column block across the batch group
                    nc.gpsimd.collective_compute(
                        "ReduceScatter",
                        mybir.AluOpType.add,
                        ins=[rs_in[jq][:]],
                        outs=[rs_out[jq][:]],
                        replica_groups=GROUPS,
                    )

                # ---- final: bias + output -----------------------------
                for i in range(2):
                    t = rsyp.tile([128, T], F32, tag="rs", name="rst")
                    for j in range(NJQ):
                        nc.sync.dma_start(
                            out=t[:, j * TQ : (j + 1) * TQ],
                            in_=rs_out[j][i * 128 : (i + 1) * 128, :],
                        )
                    nc.vector.tensor_scalar_add(
                        out=t[:], in0=t[:], scalar1=bp_sb[i][:]
                    )
                    nc.sync.dma_start(
                        out=y_d[i * 128 : (i + 1) * 128, :], in_=t[:]
                    )

    nc.compile()
    return nc


def _get_program():
    global _PROG
    if _PROG is None:
        _PROG = _build_program()
    return _PROG


def kernel(x, W_qkv, b_qkv, W_proj, b_proj):
    x = np.asarray(x, dtype=np.float32)
    W_qkv = np.asarray(W_qkv, dtype=np.float32)
    b_qkv = np.asarray(b_qkv, dtype=np.float32)
    W_proj = np.asarray(W_proj, dtype=np.float32)
    b_proj = np.asarray(b_proj, dtype=np.float32)

    nc = _get_program()

    u = np.arange(896)[None, :]
    kl = np.arange(128)[:, None]
    mask_host = (u >= kl + 384).astype(np.float32)
    ones_host = np.ones((128, 64), dtype=np.float32)
    bc2_host = np.zeros((2, 128), dtype=np.float32)
    bc2_host[0, 0:64] = 1.0
    bc2_host[1, 64:128] = 1.0

    xts = [np.ascontiguousarray(x[b].T) for b in range(B)]
    in_maps = []
    for c in range(N_CORES):
        b, g = divmod(c, 4)
        cs = slice(g * G * DH, (g + 1) * G * DH)
        r = c % 4
        in_maps.append(
            {
                "xt": xts[b],
                "wq": np.ascontiguousarray(W_qkv[:, cs]),
                "wk": np.ascontiguousarray(W_qkv[:, C:][:, cs]),
                "wv": np.ascontiguousarray(W_qkv[:, 2 * C :][:, cs]),
                "wp": np.ascontiguousarray(W_proj[cs, :]),
                "bq": np.ascontiguousarray(b_qkv[cs]).reshape(-1, 1),
                "bk": np.ascontiguousarray(b_qkv[C:][cs]).reshape(-1, 1),
                "bv": np.ascontiguousarray(b_qkv[2 * C :][cs]).reshape(1, -1),
                "bp": np.ascontiguousarray(
                    b_proj[r * 256 : (r + 1) * 256]
                ).reshape(-1, 1),
                "mask": mask_host,
                "ones": ones_host,
                "bc2": bc2_host,
            }
        )

    global _last_in_maps
    _last_in_maps = in_maps
    res = run_bass_kernel_spmd(nc, in_maps, list(range(N_CORES)))

    y = np.empty((B, T, C), dtype=np.float32)
    for b in range(B):
        yT = np.concatenate(
            [res.results[4 * b + r]["y"] for r in range(4)], axis=0
        )
        y[b] = yT.T
    return y


# revision 10
# speedup vs baseline: 1.4410x; 1.2270x over previous
"""Causal self-attention on 8 Trainium2 NeuronCores.

Reference (fp32):
    qkv = x @ W_qkv + b_qkv ; split q,k,v ; heads H=16, Dh=64
    scores = q @ k^T / sqrt(Dh), causal mask, softmax
    out = (attn @ v) re-merged ; y = out @ W_proj + b_proj

Sharding: tensor-parallel over heads x data-parallel over batch.
Core c (0..7) owns batch b = c//4 and head group g = c%4 (heads 4g..4g+3).
Each core computes q^T,k^T,v for its 4 heads from x[b]^T, runs causal
attention (scores transposed layout, exp without max-subtraction -- scores
are O(5) so fp32 exp is safe, denominator via an appended ones-column in
the V matmul), then its partial y^T = O^T @ W_proj[rows]. The 4 cores of a
batch ReduceScatter(add) the [1024, 2048] partial y^T in 4 row chunks
overlapped with the projection; each core adds its b_proj slice and
returns 4 x [64, 2048] row-slices of y^T. Host reassembles.

Matmuls run as float32r (reduced-precision fp32, 4x faster than fp32 on
the PE); end-to-end error vs the fp32 reference is ~3e-4 of max|y|.
The two heads of a pair occupy PE rows 0:64 / 64:128 so their score
matmuls execute concurrently in disjoint row groups.
"""

import numpy as np

import concourse.bacc as bacc
import concourse.mybir as mybir
import concourse.tile as tile
from concourse.bass_utils import run_bass_kernel_spmd

B = 2
T = 2048
C = 1024
H = 16
DH = 64
G = 4  # heads per core
N_CORES = 8
TQ = 512  # q-chunk width
NKT = T // 128  # k tiles per head
NJQ = T // TQ  # q chunks
NCK = C // 128  # contraction tiles over model dim
SCALE = 1.0 / np.sqrt(DH)
GROUPS = [[0, 1, 2, 3], [4, 5, 6, 7]]

F32 = mybir.dt.float32
F32R = mybir.dt.float32r
BF16 = mybir.dt.bfloat16
ATT_BF16 = True  # bf16 for the q@k^T and attn@v matmul operands
ATT_DT = BF16 if ATT_BF16 else F32R

_PROG = None


def _build_program():
    nc = bacc.Bacc(
        "TRN2", target_bir_lowering=False, debug=False, num_devices=N_CORES
    )
    xt_d = nc.dram_tensor("xt", [C, T], F32R, kind="ExternalInput").ap()
    wq_d = nc.dram_tensor("wq", [C, G * DH], F32R, kind="ExternalInput").ap()
    wk_d = nc.dram_tensor("wk", [C, G * DH], F32R, kind="ExternalInput").ap()
    wv_d = nc.dram_tensor("wv", [C, G * DH], F32R, kind="ExternalInput").ap()
    wp_d = nc.dram_tensor("wp", [G * DH, C], F32R, kind="ExternalInput").ap()
    bq_d = nc.dram_tensor("bq", [G * DH, 1], F32, kind="ExternalInput").ap()
    bk_d = nc.dram_tensor("bk", [G * DH, 1], F32, kind="ExternalInput").ap()
    bv_d = nc.dram_tensor("bv", [1, G * DH], F32, kind="ExternalInput").ap()
    bp_d = nc.dram_tensor("bp", [C // 4, 1], F32, kind="ExternalInput").ap()
    mask_d = nc.dram_tensor("mask", [128, 896], ATT_DT, kind="ExternalInput").ap()
    ones_d = nc.dram_tensor("ones", [128, 64], F32R, kind="ExternalInput").ap()
    onesb_d = nc.dram_tensor("onesb", [128, 64], ATT_DT, kind="ExternalInput").ap()
    bc2_d = nc.dram_tensor("bc2", [2, 128], F32R, kind="ExternalInput").ap()
    rs_in = [nc.dram_tensor(f"rs_in{j}", [C, TQ], F32).ap() for j in range(NJQ)]
    rs_out = [nc.dram_tensor(f"rs_out{j}", [C // 4, TQ], F32).ap() for j in range(NJQ)]
    y_d = nc.dram_tensor("y", [C // 4, T], F32, kind="ExternalOutput").ap()

    with tile.TileContext(nc) as tc:
        with (
            nc.allow_low_precision(reason="float32r matmul pipeline by design"),
            tc.tile_pool(name="ll", bufs=1) as ll,
        ):
            # ---- long-lived tiles -------------------------------------
            qT = [ll.tile([128, T], ATT_DT, tag=f"qT{p}", name=f"qT{p}") for p in range(2)]
            kT = [ll.tile([128, T], ATT_DT, tag=f"kT{p}", name=f"kT{p}") for p in range(2)]
            oT = [ll.tile([128, T], F32R, tag=f"oT{p}", name=f"oT{p}") for p in range(2)]
            vaug = [ll.tile([128, G * 65], ATT_DT, tag=f"va{t}", name=f"va{t}") for t in range(NKT)]


            mask = ll.tile([128, 896], ATT_DT, tag="mask")
            nc.sync.dma_start(out=mask[:], in_=mask_d[:])
            bc2_sb = ll.tile([2, 128], F32R, tag="bc2")
            nc.sync.dma_start(out=bc2_sb[:], in_=bc2_d[:])
            ones_sb = ll.tile([128, 64], F32R, tag="ones")
            nc.sync.dma_start(out=ones_sb[:], in_=ones_d[:])
            wp_sb = [ll.tile([128, C], F32R, tag=f"wp{p}", name=f"wp{p}") for p in range(2)]
            for p in range(2):
                nc.sync.dma_start(
                    out=wp_sb[p][:], in_=wp_d[p * 128 : (p + 1) * 128, :]
                )
            bq_sb = [ll.tile([128, 1], F32, tag=f"bq{p}", name=f"bq{p}") for p in range(2)]
            bk_sb = [ll.tile([128, 1], F32, tag=f"bk{p}", name=f"bk{p}") for p in range(2)]
            for p in range(2):
                nc.sync.dma_start(
                    out=bq_sb[p][:], in_=bq_d[p * 128 : (p + 1) * 128, :]
                )
                nc.sync.dma_start(
                    out=bk_sb[p][:], in_=bk_d[p * 128 : (p + 1) * 128, :]
                )
            bv_sb = ll.tile([1, G * DH], F32, tag="bv")
            nc.sync.dma_start(out=bv_sb[:], in_=bv_d[:])
            bp_sb = [ll.tile([128, 1], F32, tag=f"bp{i}", name=f"bp{i}") for i in range(2)]
            for i in range(2):
                nc.sync.dma_start(
                    out=bp_sb[i][:], in_=bp_d[i * 128 : (i + 1) * 128, :]
                )

            # ---- phase A: qkv projections -----------------------------
            with (
                tc.tile_pool(name="pa", bufs=1) as pa,
                tc.tile_pool(name="pamm", bufs=3, space="PSUM") as pamm,
            ):
                # bv broadcast across partitions (via ones-row matmul)
                ones_row = ll.tile([1, 128], F32R, tag="ones_row")
                nc.sync.dma_start(out=ones_row[:, 0:64], in_=ones_d[0:1, :])
                nc.sync.dma_start(out=ones_row[:, 64:128], in_=ones_d[0:1, :])
                bv_r = ll.tile([1, G * DH], F32R, tag="bvr")
                nc.vector.tensor_copy(out=bv_r[:], in_=bv_sb[:])
                bvb_ps = pamm.tile([128, G * DH], F32, tag="mm")
                bvb_sb = ll.tile([128, G * DH], F32, tag="bvb")
                nc.tensor.matmul(
                    bvb_ps[:], lhsT=ones_row[:], rhs=bv_r[:], start=True, stop=True
                )
                nc.vector.tensor_copy(out=bvb_sb[:], in_=bvb_ps[:])

                wq_sb, wk_sb, wv_sb = [], [], []
                for k in range(NCK):
                    for name, dst, src in (
                        ("q", wq_sb, wq_d),
                        ("k", wk_sb, wk_d),
                        ("v", wv_sb, wv_d),
                    ):
                        t = pa.tile([128, G * DH], F32R, tag=f"w{name}{k}", name=f"w{name}{k}")
                        nc.sync.dma_start(
                            out=t[:], in_=src[k * 128 : (k + 1) * 128, :]
                        )
                        dst.append(t)
                xt_sb = [
                    pa.tile([128, T], F32R, tag=f"xt{k}", name=f"xt{k}")
                    for k in range(NCK)
                ]
                for j in range(NJQ):
                    for k in range(NCK):
                        nc.sync.dma_start(
                            out=xt_sb[k][:, j * TQ : (j + 1) * TQ],
                            in_=xt_d[k * 128 : (k + 1) * 128, j * TQ : (j + 1) * TQ],
                        )

                for j in range(NJQ):
                    # q^T / k^T chains for this column chunk
                    for wsb, bsb, dst in ((wq_sb, bq_sb, qT), (wk_sb, bk_sb, kT)):
                        for p in range(2):
                            ps = pamm.tile([128, TQ], F32, tag="mm")
                            for k in range(NCK):
                                nc.tensor.matmul(
                                    ps[:],
                                    lhsT=wsb[k][:, p * 128 : (p + 1) * 128],
                                    rhs=xt_sb[k][:, j * TQ : (j + 1) * TQ],
                                    start=(k == 0),
                                    stop=(k == NCK - 1),
                                )
                            nc.vector.tensor_scalar_add(
                                out=dst[p][:, j * TQ : (j + 1) * TQ],
                                in0=ps[:],
                                scalar1=bsb[p][:],
                            )
                    # v tiles covered by this column chunk
                    for t in range(4 * j, 4 * j + 4):
                        ps = pamm.tile([128, G * DH], F32, tag="mm")
                        for k in range(NCK):
                            nc.tensor.matmul(
                                ps[:],
                                lhsT=xt_sb[k][:, t * 128 : (t + 1) * 128],
                                rhs=wv_sb[k][:],
                                start=(k == 0),
                                stop=(k == NCK - 1),
                            )
                        va = vaug[t].rearrange("p (h x) -> p h x", x=65)
                        nc.vector.tensor_add(
                            out=va[:, :, 0:64],
                            in0=ps[:].rearrange("p (h x) -> p h x", x=64),
                            in1=bvb_sb[:].rearrange("p (h x) -> p h x", x=64),
                        )
                        nc.sync.dma_start(
                            out=va[:, :, 64:65],
                            in_=onesb_d[:, 0:G].rearrange("p (h x) -> p h x", x=1),
                        )

            # ---- phases B..D: attention, normalize, projection, RS ----
            # jq-outer so that normalize/proj/ReduceScatter pipeline per
            # 512-column block while later blocks still compute.
            with (
                tc.tile_pool(name="dt", bufs=4) as dtp,
                tc.tile_pool(name="rp", bufs=2) as rpp,
                tc.tile_pool(name="es", bufs=6) as esp,
                tc.tile_pool(name="oc", bufs=3) as ocp,
                tc.tile_pool(name="rsy", bufs=2) as rsyp,
                tc.tile_pool(name="ps0", bufs=2, space="PSUM") as sp0,
                tc.tile_pool(name="ps1", bufs=2, space="PSUM") as sp1,
                tc.tile_pool(name="ov", bufs=2, space="PSUM") as ovp,
                tc.tile_pool(name="px", bufs=2, space="PSUM") as pxp,
            ):
                for jq in range(NJQ):
                    kmax = 4 * jq + 4
                    den4 = dtp.tile([4, TQ], F32, tag="den4", name="den4")
                    for p in range(2):
                        ov = [
                            ovp.tile([65, TQ], F32, tag="ov", name="ovA"),
                            ovp.tile([65, TQ], F32, tag="ov", name="ovB"),
                        ]
                        spool = (sp0, sp1)

                        def emit_v(kt, es_pair):
                            va = vaug[kt].rearrange("p (h x) -> p h x", x=65)
                            for half in range(2):
                                nc.tensor.matmul(
                                    ov[half][:],
                                    lhsT=va[:, 2 * p + half, :],
                                    rhs=es_pair[half][:],
                                    start=(kt == 0),
                                    stop=(kt == kmax - 1),
                                )

                        prev = None
                        for kt in range(kmax):
                            es_pair = []
                            for half in range(2):
                                r = 64 * half
                                sps = spool[half].tile(
                                    [128, TQ], F32, tag="s", name="sps"
                                )
                                nc.tensor.matmul(
                                    sps[:],
                                    lhsT=kT[p][
                                        r : r + 64, kt * 128 : (kt + 1) * 128
                                    ],
                                    rhs=qT[p][r : r + 64, jq * TQ : (jq + 1) * TQ],
                                    start=True,
                                    stop=True,
                                )
                                es = esp.tile([128, TQ], ATT_DT, tag="es", name="es")
                                nc.scalar.activation(
                                    out=es[:],
                                    in_=sps[:],
                                    func=mybir.ActivationFunctionType.Exp,
                                    scale=SCALE,
                                )
                                if kt >= 4 * jq:
                                    off = 384 - (128 * kt - TQ * jq)
                                    nc.vector.tensor_mul(
                                        out=es[:],
                                        in0=es[:],
                                        in1=mask[:, off : off + TQ],
                                    )
                                es_pair.append(es)
                            if prev is not None:
                                emit_v(*prev)
                            prev = (kt, es_pair)
                        emit_v(*prev)
                        # epilogue: move unnormalized O and denominators out
                        for half in range(2):
                            nc.vector.tensor_copy(
                                out=oT[p][
                                    64 * half : 64 * half + 64,
                                    jq * TQ : (jq + 1) * TQ,
                                ],
                                in_=ov[half][0:64, :],
                            )
                            dt_t = dtp.tile([1, TQ], F32, tag="dt", name="dt")
                            nc.vector.tensor_copy(
                                out=dt_t[:], in_=ov[half][64:65, :]
                            )
                            nc.sync.dma_start(
                                out=den4[2 * p + half : 2 * p + half + 1, :],
                                in_=dt_t[:],
                            )

                    # normalize this column block (both pairs)
                    rec4 = dtp.tile([4, TQ], F32R, tag="rec4", name="rec4")
                    nc.vector.reciprocal(out=rec4[:], in_=den4[:])
                    for p in range(2):
                        rp_t = rpp.tile([2, TQ], F32R, tag="rp", name="rp")
                        nc.sync.dma_start(
                            out=rp_t[:], in_=rec4[2 * p : 2 * p + 2, :]
                        )
                        recb = pxp.tile([128, TQ], F32, tag="x", name="recb")
                        nc.tensor.matmul(
                            recb[:],
                            lhsT=bc2_sb[:],
                            rhs=rp_t[:],
                            start=True,
                            stop=True,
                        )
                        dst = oT[p][:, jq * TQ : (jq + 1) * TQ]
                        nc.vector.tensor_mul(out=dst, in0=dst, in1=recb[:])

                    # projection for this column block
                    for et in range(C // 128):
                        ps = pxp.tile([128, TQ], F32, tag="x", name="pmm")
                        for p in range(2):
                            nc.tensor.matmul(
                                ps[:],
                                lhsT=wp_sb[p][:, et * 128 : (et + 1) * 128],
                                rhs=oT[p][:, jq * TQ : (jq + 1) * TQ],
                                start=(p == 0),
                                stop=(p == 1),
                            )
                        o = ocp.tile([128, TQ], F32, tag="oc", name="oc")
                        nc.vector.tensor_copy(out=o[:], in_=ps[:])
                        nc.sync.dma_start(
                            out=rs_in[jq][et * 128 : (et + 1) * 128, :], in_=o[:]
                        )
                    # reduce-scatter this column block across the batch group
                    nc.gpsimd.collective_compute(
                        "ReduceScatter",
                        mybir.AluOpType.add,
                        ins=[rs_in[jq][:]],
                        outs=[rs_out[jq][:]],
                        replica_groups=GROUPS,
                    )

                # ---- final: bias + output -----------------------------
                for i in range(2):
                    t = rsyp.tile([128, T], F32, tag="rs", name="rst")
                    for j in range(NJQ):
                        nc.sync.dma_start(
                            out=t[:, j * TQ : (j + 1) * TQ],
                            in_=rs_out[j][i * 128 : (i + 1) * 128, :],
                        )
                    nc.vector.tensor_scalar_add(
                        out=t[:], in0=t[:], scalar1=bp_sb[i][:]
                    )
                    nc.sync.dma_start(
                        out=y_d[i * 128 : (i + 1) * 128, :], in_=t[:]
                    )

    nc.compile()
    return nc


def _get_program():
    global _PROG
    if _PROG is None:
        _PROG = _build_program()
    return _PROG


def kernel(x, W_qkv, b_qkv, W_proj, b_proj):
    x = np.asarray(x, dtype=np.float32)
    W_qkv = np.asarray(W_qkv, dtype=np.float32)
    b_qkv = np.asarray(b_qkv, dtype=np.float32)
    W_proj = np.asarray(W_proj, dtype=np.float32)
    b_proj = np.asarray(b_proj, dtype=np.float32)

    nc = _get_program()

    import ml_dtypes

    att_np = ml_dtypes.bfloat16 if ATT_BF16 else np.float32
    u = np.arange(896)[None, :]
    kl = np.arange(128)[:, None]
    mask_host = (u >= kl + 384).astype(att_np)
    ones_host = np.ones((128, 64), dtype=np.float32)
    onesb_host = np.ones((128, 64), dtype=att_np)
    bc2_host = np.zeros((2, 128), dtype=np.float32)
    bc2_host[0, 0:64] = 1.0
    bc2_host[1, 64:128] = 1.0

    xts = [np.ascontiguousarray(x[b].T) for b in range(B)]
    in_maps = []
    for c in range(N_CORES):
        b, g = divmod(c, 4)
        cs = slice(g * G * DH, (g + 1) * G * DH)
        r = c % 4
        in_maps.append(
            {
                "xt": xts[b],
                "wq": np.ascontiguousarray(W_qkv[:, cs]),
                "wk": np.ascontiguousarray(W_qkv[:, C:][:, cs]),
                "wv": np.ascontiguousarray(W_qkv[:, 2 * C :][:, cs]),
                "wp": np.ascontiguousarray(W_proj[cs, :]),
                "bq": np.ascontiguousarray(b_qkv[cs]).reshape(-1, 1),
                "bk": np.ascontiguousarray(b_qkv[C:][cs]).reshape(-1, 1),
                "bv": np.ascontiguousarray(b_qkv[2 * C :][cs]).reshape(1, -1),
                "bp": np.ascontiguousarray(
                    b_proj[r * 256 : (r + 1) * 256]
                ).reshape(-1, 1),
                "mask": mask_host,
                "ones": ones_host,
                "onesb": onesb_host,
                "bc2": bc2_host,
            }
        )

    global _last_in_maps
    _last_in_maps = in_maps
    res = run_bass_kernel_spmd(nc, in_maps, list(range(N_CORES)))

    y = np.empty((B, T, C), dtype=np.float32)
    for b in range(B):
        yT = np.concatenate(
            [res.results[4 * b + r]["y"] for r in range(4)], axis=0
        )
        y[b] = yT.T
    return y
